# revision 38
# baseline (speedup 1.0000x reference)
"""Trainium2 Bass kernel for quantized multi-head attention (ViT-shape).

Computation (per reference):
  q/k/v = x @ W{q,k,v}.T ; per-head scores = (q k^T) * D^-0.5 ;
  fake_quant_per_head(scores) ; softmax ; out = attn @ v ;
  fake_quant_per_head(out) ; merge heads ; out @ Wo.T + bo.

Sharding: data-parallel over batch, 8 images per core on 8 NeuronCores.

Key device-side design (per core, 8 images = 1576 tokens, 4 chunks of 2
images):
  - All weights host-transposed to [d_in, d_out]; alpha/s_attn folded into
    Wq; s_out folded into Wo; quant lo offset folded into the output bias.
  - q,k feature-major qT/kT [768, t]; scores transposed ST[j, i] with the
    per-(head,jtile) PSUM tile covering BOTH images of the chunk [jl, 394]
    so the fake-quant clip (DVE/GPSIMD tensor_scalar -> int16 trunc) and
    exp (ACT, scale=s_attn[h]) run as one wide instruction each.
  - softmax denominator comes free from the P@V matmul: v stored with an
    extra per-head column holding s_out[h]; PV column 64 is s_out*sum(E).
  - out-quant batched over all 12 heads per (im, it): reciprocal, then
    tensor_tensor mult with a broadcast inv AP, tensor_tensor min/max
    against prebuilt per-head hi/lo constant tiles, int16-convert trunc.
  - Oq (integer-valued int16) is PE-transposed (int16 identity) into
    128-col-aligned PSUM blocks, assembled to feature-major OT (bf16),
    then the Wo matmul + bias-add (gpsimd tensor_scalar) + DMA out.
  - elementwise work is spread across DVE / GPSIMD / ACT so the kernel
    stays tensor-engine-bound.
"""

import os
import numpy as np

B, N, D, H = 64, 197, 768, 12
DH = D // H  # 64
NCORES = 8
BPC = B // NCORES          # 8 images per core
T = BPC * N                # 1576 tokens per core
IMGS_PER_CHUNK = 2
NCHUNK = BPC // IMGS_PER_CHUNK  # 4
TC = IMGS_PER_CHUNK * N    # 394 tokens per chunk
KT = D // 128              # 6 d-tiles
OT = D // 128              # 6 o-tiles
Q_LEVELS = 255

_RUNNER_CACHE = {}


def _head_off(h):
    # per-image wide PV psum [128, 1024] (2 banks): heads 0-6 in bank 0,
    # heads 7-11 in bank 1 (a 65-wide block may not cross a 512-f32 bank).
    return 65 * h if h < 7 else 512 + 65 * (h - 7)


def _build_program(hi_s_attn, lo_s_attn, s_attn, hi_s_out, lo_s_out, s_out, variant,
                   reps=1):
    import concourse.bass as bass
    import concourse.bacc as bacc
    import concourse.mybir as mybir
    from concourse.tile import TileContext

    f32 = mybir.dt.float32
    bf16 = mybir.dt.bfloat16
    i16 = mybir.dt.int16

    if variant == "f32":
        attn_dt = f32
    elif variant == "bf16":
        attn_dt = bf16
    else:
        raise ValueError(variant)
    w_dt = attn_dt

    nc = bacc.Bacc("TRN2", target_bir_lowering=False, debug=False)

    xT_d = nc.dram_tensor("xT", [D, T], w_dt, kind="ExternalInput").ap()
    wq_d = nc.dram_tensor("wqts", [D, D], w_dt, kind="ExternalInput").ap()
    wk_d = nc.dram_tensor("wkt", [D, D], w_dt, kind="ExternalInput").ap()
    wv_d = nc.dram_tensor("wvt", [D, D], w_dt, kind="ExternalInput").ap()
    wo_d = nc.dram_tensor("wots", [D, D], w_dt, kind="ExternalInput").ap()
    bo_d = nc.dram_tensor("bof", [D], f32, kind="ExternalInput").ap()
    id_d = nc.dram_tensor("ident", [128, 128], attn_dt,
                          kind="ExternalInput").ap()
    out_d = nc.dram_tensor("outT", [D, T], f32, kind="ExternalOutput").ap()

    Exp = mybir.ActivationFunctionType.Exp
    Ident = mybir.ActivationFunctionType.Identity
    A = mybir.AluOpType

    with TileContext(nc) as tc:
        with (
            tc.tile_pool(name="const", bufs=1) as cpool,
            tc.tile_pool(name="sb", bufs=2) as sb,
            tc.tile_pool(name="ps", bufs=2, space="PSUM") as ps,
        ):
            # ---- resident constants ----
            wq_sb, wk_sb, wv_sb, wo_sb = [], [], [], []
            for (lst, dram, nm) in ((wq_sb, wq_d, "wq"), (wk_sb, wk_d, "wk"),
                                    (wv_sb, wv_d, "wv"), (wo_sb, wo_d, "wo")):
                for k in range(KT):
                    t = cpool.tile([128, D], w_dt, name=f"{nm}{k}")
                    nc.sync.dma_start(out=t,
                                      in_=dram[128 * k:128 * (k + 1), :])
                    lst.append(t)
            bo_sb = cpool.tile([128, OT], f32, name="bo")
            for k in range(OT):
                nc.sync.dma_start(
                    out=bo_sb[:, k:k + 1],
                    in_=bo_d[128 * k:128 * (k + 1)].rearrange("(p o) -> p o", o=1),
                )
            ident = cpool.tile([128, 128], attn_dt, name="ident")
            nc.sync.dma_start(out=ident, in_=id_d)
            # per-head constants: s_out column for v augmentation; hi/lo
            # out-quant clip bounds replicated over each head's 64 dims.
            scol = cpool.tile([128, H], attn_dt, name="scol")
            hi_full = cpool.tile([128, D], f32, name="hifull")
            lo_full = cpool.tile([128, D], f32, name="lofull")
            for h in range(H):
                nc.gpsimd.memset(scol[:, h:h + 1], float(s_out[h]))
                nc.gpsimd.memset(hi_full[:, 64 * h:64 * (h + 1)],
                                 float(hi_s_out[h]))
                nc.gpsimd.memset(lo_full[:, 64 * h:64 * (h + 1)],
                                 float(lo_s_out[h]))

            import contextlib
            rep_ctx = tc.For_i(0, reps, 1) if reps > 1 else contextlib.nullcontext()
            with rep_ctx:
                _emit_body(nc, tc, sb, ps, locals())
    nc.compile()
    return nc


def _emit_body(nc, tc, sb, ps, env):
    (xT_d, out_d, wq_sb, wk_sb, wv_sb, wo_sb, bo_sb, ident) = (
        env["xT_d"], env["out_d"], env["wq_sb"], env["wk_sb"], env["wv_sb"],
        env["wo_sb"], env["bo_sb"], env["ident"])
    (w_dt, attn_dt, f32, i16) = env["w_dt"], env["attn_dt"], env["f32"], env["i16"]
    (scol, hi_full, lo_full) = env["scol"], env["hi_full"], env["lo_full"]
    (hi_s_attn, lo_s_attn, s_attn) = env["hi_s_attn"], env["lo_s_attn"], env["s_attn"]
    (hi_s_out, lo_s_out, s_out) = env["hi_s_out"], env["lo_s_out"], env["s_out"]
    Exp, Ident, A = env["Exp"], env["Ident"], env["A"]

    st = [dict() for _ in range(NCHUNK)]  # per-chunk tile state

    def gen_A(c):
        """xc DMA + q/k/v projections; yields after each psum group."""
        c0 = TC * c
        xc = sb.tile([128, KT * TC], w_dt, name=f"xc{c}", tag="xc")
        xeng = nc.scalar if c == 0 else nc.sync
        for k in range(KT):
            xeng.dma_start(
                out=xc[:, TC * k:TC * (k + 1)],
                in_=xT_d[128 * k:128 * (k + 1), c0:c0 + TC],
            )
        qc = sb.tile([128, OT * TC], attn_dt, name=f"qc{c}", tag="qc")
        kc = sb.tile([128, OT * TC], attn_dt, name=f"kc{c}", tag="kc")
        st[c].update(qc=qc, kc=kc, vaug=[])
        for (wsb, dst) in ((wq_sb, qc), (wk_sb, kc)):
            for o in range(OT):
                pj = ps.tile([128, TC], f32, name=f"pj{c}{o}", tag="proj")
                for k in range(KT):
                    nc.tensor.matmul(
                        pj,
                        lhsT=wsb[k][:, 128 * o:128 * (o + 1)],
                        rhs=xc[:, TC * k:TC * (k + 1)],
                        start=(k == 0), stop=(k == KT - 1),
                    )
                nc.scalar.activation(dst[:, TC * o:TC * (o + 1)], pj, Ident)
                yield
        vaug = st[c]["vaug"]
        for im in range(IMGS_PER_CHUNK):
            for tt in range(2):
                tl = 128 if tt == 0 else N - 128
                va = sb.tile([128, H * 65], attn_dt,
                             name=f"va{c}{im}{tt}", tag="vaug", bufs=10)
                vav = va.rearrange("p (h c) -> p h c", c=65)
                for oc in range(2):
                    vp = ps.tile([128, 384], f32,
                                 name=f"vp{c}{im}{tt}{oc}", tag="proj")
                    for k in range(KT):
                        nc.tensor.matmul(
                            vp[:tl],
                            lhsT=xc[:, TC * k + N * im + 128 * tt:
                                    TC * k + N * im + 128 * tt + tl],
                            rhs=wv_sb[k][:, 384 * oc:384 * (oc + 1)],
                            start=(k == 0), stop=(k == KT - 1),
                        )
                    if oc == 0:
                        nc.vector.tensor_copy(
                            vav[:tl, 0:6, 0:64],
                            vp[:tl].rearrange("p (h c) -> p h c", c=64),
                        )
                    else:
                        nc.scalar.activation(
                            vav[:tl, 6:12, 0:64],
                            vp[:tl].rearrange("p (h c) -> p h c", c=64),
                            Ident,
                        )
                nc.gpsimd.tensor_copy(vav[:tl, :, 64:65],
                                      scol[:tl].unsqueeze(2))
                vaug.append(va)
                yield

    def emit_pv(c, pv_t, h, im, it):
        il = 128 if it == 0 else N - 128
        off = _head_off(h)
        for jt in range(2):
            jl = 128 if jt == 0 else N - 128
            nc.tensor.matmul(
                pv_t[:il, off:off + 65],
                lhsT=st[c]["efs"][h][:jl,
                                     TC * jt + N * im + 128 * it:
                                     TC * jt + N * im + 128 * it + il],
                rhs=st[c]["vaug"][2 * im + jt].rearrange(
                    "p (h c) -> p h c", c=65)[:jl, h, :],
                start=(jt == 0), stop=(jt == 1),
            )

    def gen_B(c):
        """scores + fake-quant + exp, PV(it=0) at lag 2; yields per head."""
        qc, kc = st[c]["qc"], st[c]["kc"]
        efs = []
        st[c]["efs"] = efs
        pv0 = [ps.tile([128, 1024], f32, name=f"pv{c}{im}0", tag="pv")
               for im in range(IMGS_PER_CHUNK)]
        st[c]["pv0"] = pv0
        LAG = 2
        for h in range(H):
            o, row = h // 2, (h % 2) * 64
            q16 = sb.tile([128, 2 * TC], i16,
                          name=f"q16{c}{h}", tag="q16", bufs=4)
            for jt in range(2):
                jl = 128 if jt == 0 else N - 128
                sp = ps.tile([128, TC], f32, name=f"sp{c}{h}{jt}", tag="st")
                for im in range(IMGS_PER_CHUNK):
                    base = TC * o + N * im
                    nc.tensor.matmul(
                        sp[:jl, N * im:N * (im + 1)],
                        lhsT=kc[row:row + 64,
                                base + 128 * jt:base + 128 * jt + jl],
                        rhs=qc[row:row + 64, base:base + N],
                        start=True, stop=True,
                    )
                nc.vector.tensor_scalar(
                    out=q16[:jl, TC * jt:TC * (jt + 1)], in0=sp[:jl],
                    scalar1=float(hi_s_attn[h]),
                    scalar2=float(lo_s_attn[h]),
                    op0=A.min, op1=A.max,
                )
            # single exp over both j-tiles; rows 69-127 of the jt=1 half
            # hold bounded stale int16s (never read downstream).
            ef = sb.tile([128, 2 * TC], attn_dt,
                         name=f"ef{c}{h}", tag="ef", bufs=14)
            nc.scalar.activation(ef, q16, Exp, scale=float(s_attn[h]))
            efs.append(ef)
            if h >= LAG:
                for im in range(IMGS_PER_CHUNK):
                    emit_pv(c, pv0[im], h - LAG, im, 0)
            yield
        for h in range(H - LAG, H):
            for im in range(IMGS_PER_CHUNK):
                emit_pv(c, pv0[im], h, im, 0)

    def emit_outquant(c, pv_t, im, it):
        il = 128 if it == 0 else N - 128
        bank0 = pv_t[:, 0:65 * 7].rearrange("p (h c) -> p h c", c=65)
        bank1 = pv_t[:, 512:512 + 65 * 5].rearrange("p (h c) -> p h c", c=65)
        inv = sb.tile([128, H], f32, name=f"inv{c}{im}{it}",
                      tag="inv", bufs=4)
        nc.vector.reciprocal(inv[:il, 0:7], bank0[:il, :, 64])
        nc.vector.reciprocal(inv[:il, 7:12], bank1[:il, :, 64])
        oqt = sb.tile([128, D], f32, name=f"oqt{c}{im}{it}",
                      tag="oqt", bufs=4)
        oqt_v = oqt.rearrange("p (h c) -> p h c", c=64)
        nc.vector.tensor_tensor(
            out=oqt_v[:il, 0:7, :], in0=bank0[:il, :, 0:64],
            in1=inv[:il, 0:7].unsqueeze(2).broadcast_to((il, 7, 64)),
            op=A.mult,
        )
        nc.vector.tensor_tensor(
            out=oqt_v[:il, 7:12, :], in0=bank1[:il, :, 0:64],
            in1=inv[:il, 7:12].unsqueeze(2).broadcast_to((il, 5, 64)),
            op=A.mult,
        )
        oqi = sb.tile([128, D], i16, name=f"oqi{c}{im}{it}",
                      tag="oqi", bufs=6)
        for h in range(H):
            eng = nc.gpsimd if h < 6 else nc.vector
            eng.tensor_scalar(
                out=oqt[:il, 64 * h:64 * (h + 1)],
                in0=oqt[:il, 64 * h:64 * (h + 1)],
                scalar1=float(hi_s_out[h]), scalar2=float(lo_s_out[h]),
                op0=A.min, op1=A.max,
            )
        nc.vector.tensor_copy(oqi[:il], oqt[:il])
        oqf = sb.tile([128, D], attn_dt, name=f"oqf{c}{im}{it}",
                      tag="oqf", bufs=8)
        nc.vector.tensor_copy(oqf[:il], oqi[:il])
        st[c].setdefault("oqf", {})[(im, it)] = oqf

    def emit_transposes(c):
        oqf = st[c]["oqf"]
        otc = sb.tile([128, KT * TC], attn_dt, name=f"otc{c}", tag="otc",
                      bufs=1)
        st[c]["otc"] = otc
        for k in range(KT):
            tp = ps.tile([128, 512], attn_dt, name=f"tp{c}{k}", tag="proj")
            for it in range(2):
                il = 128 if it == 0 else N - 128
                for im in range(IMGS_PER_CHUNK):
                    nc.tensor.transpose(
                        tp[:, 128 * (2 * it + im):128 * (2 * it + im) + il],
                        oqf[(im, it)][:il, 128 * k:128 * (k + 1)],
                        ident[:il, :il],
                    )
            tp_v = tp.rearrange("p (b c) -> p b c", c=128)
            otc_v = otc[:, TC * k:TC * (k + 1)].rearrange(
                "p (i c) -> p i c", c=N)
            nc.vector.tensor_copy(otc_v[:, :, 0:128], tp_v[:, 0:2, :])
            nc.vector.tensor_copy(otc_v[:, :, 128:N], tp_v[:, 2:4, 0:N - 128])

    def emit_outproj(c):
        c0 = TC * c
        otc = st[c]["otc"]
        for o in range(OT):
            op_ = ps.tile([128, TC], f32, name=f"op{c}{o}", tag="proj")
            for k in range(KT):
                nc.tensor.matmul(
                    op_,
                    lhsT=wo_sb[k][:, 128 * o:128 * (o + 1)],
                    rhs=otc[:, TC * k:TC * (k + 1)],
                    start=(k == 0), stop=(k == KT - 1),
                )
            osb = sb.tile([128, TC], f32, name=f"osb{c}{o}", tag="osb",
                          bufs=3)
            nc.scalar.activation(osb, op_, Ident, bias=bo_sb[:, o:o + 1])
            nc.sync.dma_start(
                out=out_d[128 * o:128 * (o + 1), c0:c0 + TC], in_=osb
            )

    def stage_CxD(c, cprev):
        """C(c) with D(cprev) slotted into the out-quant chain shadows."""
        for im in range(IMGS_PER_CHUNK):
            emit_outquant(c, st[c]["pv0"][im], im, 0)
        if cprev is not None:
            emit_transposes(cprev)
        pv1 = [ps.tile([128, 1024], f32, name=f"pv{c}{im}1", tag="pv")
               for im in range(IMGS_PER_CHUNK)]
        for im in range(IMGS_PER_CHUNK):
            for h in range(H):
                emit_pv(c, pv1[im], h, im, 1)
        if cprev is not None:
            emit_outproj(cprev)
        for im in range(IMGS_PER_CHUNK):
            emit_outquant(c, pv1[im], im, 1)

    def stage_BxA(c, cnext):
        """B(c) with A(cnext)'s projection groups interleaved per head."""
        gb = gen_B(c)
        ga = gen_A(cnext) if cnext is not None else None
        n_a = 0
        for h in range(H):
            next(gb, None)
            if ga is not None:
                pulls = 2 if h < 4 else 1
                for _ in range(pulls):
                    if next(ga, True) is True:
                        ga = None
                        break
        for _ in gb:
            pass
        if ga is not None:
            for _ in ga:
                pass

    # software pipeline: next-chunk projection groups are interleaved
    # between score/PV head groups (keeps the PE stream dense so HAM stays
    # at full clock), and the previous chunk's transpose/out-proj slots
    # into the out-quant chain shadows.
    def stage_A(c):
        for _ in gen_A(c):
            pass

    def stage_B(c):
        for _ in gen_B(c):
            pass

    def stage_C(c):
        stage_CxD(c, None)

    def stage_D(c):
        emit_transposes(c)
        emit_outproj(c)

    stage_A(0)
    stage_B(0)
    stage_A(1)
    stage_C(0)
    stage_B(1)
    stage_A(2)
    stage_CxD(1, 0)
    stage_B(2)
    stage_A(3)
    stage_CxD(2, 1)
    stage_B(3)
    stage_CxD(3, 2)
    stage_D(3)


def _prepare_host_inputs(x, Wq, Wk, Wv, Wo, bo,
                         qmin_attn, qmax_attn, qmin_out, qmax_out, variant):
    """Returns (in_maps list per core, qparam tuple)."""
    f = np.float32
    alpha = np.float32(D ** -0.5)
    s_attn = ((qmax_attn - qmin_attn) / Q_LEVELS).astype(f)
    s_out = ((qmax_out - qmin_out) / Q_LEVELS).astype(f)
    hi_s_attn = (qmax_attn / s_attn).astype(f)
    lo_s_attn = (qmin_attn / s_attn).astype(f)
    hi_s_out = (qmax_out / s_out).astype(f)
    lo_s_out = (qmin_out / s_out).astype(f)

    head_of_o = np.arange(D) // DH
    wqts = np.ascontiguousarray(
        (Wq * (alpha / s_attn[head_of_o])[:, None]).T).astype(f)
    wkt = np.ascontiguousarray(Wk.T).astype(f)
    wvt = np.ascontiguousarray(Wv.T).astype(f)
    wots = np.ascontiguousarray((Wo * s_out[head_of_o][None, :]).T).astype(f)
    bof = (bo + Wo @ qmin_out[head_of_o]).astype(f)

    if variant == "bf16":
        import ml_dtypes
        adt = ml_dtypes.bfloat16
    else:
        adt = f
    ident = np.eye(128, dtype=adt)

    in_maps = []
    for i in range(NCORES):
        xs = np.ascontiguousarray(
            x[BPC * i:BPC * (i + 1)].reshape(T, D).T).astype(adt)
        in_maps.append(dict(xT=xs, wqts=wqts.astype(adt), wkt=wkt.astype(adt),
                            wvt=wvt.astype(adt), wots=wots.astype(adt),
                            bof=bof, ident=ident))
    qparams = (hi_s_attn, lo_s_attn, s_attn, hi_s_out, lo_s_out, s_out)
    return in_maps, qparams


class _Runner:
    """Compiled SPMD executable over 8 cores (PJRT path, jit cached)."""

    def __init__(self, nc):
        import jax
        import concourse.mybir as mybir
        from concourse import bass2jax
        from jax.sharding import Mesh, PartitionSpec
        from jax.experimental.shard_map import shard_map

        bass2jax.install_neuronx_cc_hook()
        self.nc = nc
        assert nc.dbg_addr is None
        partition_name = (nc.partition_id_tensor.name
                          if nc.partition_id_tensor else None)

        in_names, out_names, out_avals, zero_outs = [], [], [], []
        for alloc in nc.m.functions[0].allocations:
            if not isinstance(alloc, mybir.MemoryLocationSet):
                continue
            name = alloc.memorylocations[0].name
            if alloc.kind == "ExternalInput":
                if name != partition_name:
                    in_names.append(name)
            elif alloc.kind == "ExternalOutput":
                shape = tuple(alloc.tensor_shape)
                dtype = mybir.dt.np(alloc.dtype)
                out_names.append(name)
                out_avals.append(jax.core.ShapedArray(shape, dtype))
                zero_outs.append(np.zeros(shape, dtype))
        self.in_names, self.out_names = in_names, out_names
        self.out_avals, self.zero_outs = out_avals, zero_outs
        n_params, n_outs = len(in_names), len(out_avals)
        all_names = list(in_names) + list(out_names)
        if partition_name is not None:
            all_names.append(partition_name)
        all_names = tuple(all_names)

        def _body(*args):
            operands = list(args)
            if partition_name is not None:
                operands.append(bass2jax.partition_id_tensor())
            outs = bass2jax._bass_exec_p.bind(
                *operands,
                out_avals=tuple(out_avals),
                in_names=all_names,
                out_names=tuple(out_names),
                lowering_input_output_aliases=(),
                sim_require_finite=True,
                sim_require_nnan=True,
                nc=nc,
            )
            return tuple(outs)

        devices = jax.devices()[:NCORES]
        mesh = Mesh(np.asarray(devices), ("core",))
        self.mesh = mesh
        self.spec = PartitionSpec("core")
        self.sharded = jax.jit(
            shard_map(_body, mesh=mesh,
                      in_specs=(PartitionSpec("core"),) * (n_params + n_outs),
                      out_specs=(PartitionSpec("core"),) * n_outs,
                      check_rep=False),
            donate_argnums=tuple(range(n_params, n_params + n_outs)),
            keep_unused=True,
        )
        import jax.numpy as jnp
        from jax.sharding import NamedSharding
        zshardings = tuple(NamedSharding(mesh, self.spec) for _ in zero_outs)
        zshapes = [(NCORES * z.shape[0], *z.shape[1:]) for z in zero_outs]
        zdtypes = [z.dtype for z in zero_outs]
        self.zeros_fn = jax.jit(
            lambda: tuple(jnp.zeros(s, d) for s, d in zip(zshapes, zdtypes)),
            out_shardings=zshardings,
        )

    def device_put_inputs(self, concat_in):
        import jax
        from jax.sharding import NamedSharding
        sh = NamedSharding(self.mesh, self.spec)
        return [jax.device_put(a, sh) for a in concat_in]

    def concat_inputs(self, in_maps):
        return [np.concatenate([np.asarray(m[name]) for m in in_maps], axis=0)
                for name in self.in_names]

    def run_raw(self, concat_in):
        return self.sharded(*concat_in, *self.zeros_fn())

    def run(self, in_maps):
        out_arrs = self.run_raw(self.concat_inputs(in_maps))
        return [
            {name: np.asarray(out_arrs[i]).reshape(
                NCORES, *self.out_avals[i].shape)[c]
             for i, name in enumerate(self.out_names)}
            for c in range(NCORES)
        ]


def get_runner(qparams, variant):
    key = (variant,) + tuple(p.tobytes() for p in qparams)
    if key not in _RUNNER_CACHE:
        _RUNNER_CACHE[key] = _Runner(_build_program(*qparams, variant))
    return _RUNNER_CACHE[key]


def kernel(x, Wq, Wk, Wv, Wo, bo, qmin_attn, qmax_attn, qmin_out, qmax_out):
    variant = os.environ.get("KVAR", "bf16")
    in_maps, qparams = _prepare_host_inputs(
        np.asarray(x, np.float32), np.asarray(Wq, np.float32),
        np.asarray(Wk, np.float32), np.asarray(Wv, np.float32),
        np.asarray(Wo, np.float32), np.asarray(bo, np.float32),
        np.asarray(qmin_attn, np.float32), np.asarray(qmax_attn, np.float32),
        np.asarray(qmin_out, np.float32), np.asarray(qmax_out, np.float32),
        variant,
    )
    runner = get_runner(qparams, variant)
    results = runner.run(in_maps)
    out = np.empty((B, N, D), np.float32)
    for i in range(NCORES):
        out[BPC * i:BPC * (i + 1)] = results[i]["outT"].T.reshape(BPC, N, D)
    kernel.last_runner = runner
    kernel.last_in_maps = in_maps
    return out
